# revision 7
# baseline (speedup 1.0000x reference)
"""Causal attention layer (B=4, N=2048, C=1024, H=16, D=64) on 8 TRN2 NeuronCores.

Sharding: core c -> (batch b = c//2, head-group g = c%2 of 8 heads).
Each core computes, for its (b, g):
  qkvT  = wqkvT_g.T-contract(x_b)      (q,k transposed [o,n]; v transposed, then
                                        PE-transposed to natural [n,o] + ones col)
  S_T   = kT.T @ qT  (pre-transposed scores [kn, qn]), exp on ScalarE -> P_T bf16
  oT    = [v|1].T @ P_T                (attn out transposed + softmax denominators)
  attn_outT = oT[0:64] * (1/oT[64])    (normalize, rank-1 broadcast via PE)
  out_part  = attn_outT.T-contract(projT_g)   ([n, o] partial over 512 channels)
Host sums the two head-group partials per batch and adds proj_b.

All matmuls run as float32r (full PE rate at N=512, ~1.5e-4 rel precision);
attention probabilities P_T and v_ext are bf16.
"""
import sys

sys.path.insert(0, "/opt/trn_rl_repo")

import numpy as np

import concourse.bass as bass  # noqa: F401  (bass types via bacc)
import concourse.tile as tile
from concourse import bacc, mybir
from concourse.bass_utils import run_bass_kernel_spmd

F32 = mybir.dt.float32
F32R = mybir.dt.float32r
BF16 = mybir.dt.bfloat16
EXP = mybir.ActivationFunctionType.Exp

B, N, C, H, D = 4, 2048, 1024, 16, 64
G = 8            # heads per core
GC = G * D       # 512 channels per core
NT = N // 128    # 16 row tiles
NS = N // 512    # 4 row supers
CK = C // 128    # 8 contraction chunks

_cache = {}


def _build_nc():
    nc = bacc.Bacc("TRN2", target_bir_lowering=False, debug=False)

    xT_d = nc.dram_tensor("xT", [C, N], F32R, kind="ExternalInput")
    wqkvT_d = nc.dram_tensor("wqkvT", [C, 3 * GC], F32R, kind="ExternalInput")
    projT_d = nc.dram_tensor("projT", [GC, C], F32R, kind="ExternalInput")
    masks_d = nc.dram_tensor("masks", [128, 4, 512], BF16, kind="ExternalInput")
    ident_d = nc.dram_tensor("ident", [128, 128], BF16, kind="ExternalInput")
    ones_d = nc.dram_tensor("ones", [128, 64], F32R, kind="ExternalInput")
    out_d = nc.dram_tensor("out", [N, C], F32, kind="ExternalOutput")

    with tile.TileContext(nc) as tc:
        with (
            tc.tile_pool(name="consts", bufs=1) as consts,
            tc.tile_pool(name="qk", bufs=1) as qk_pool,
            tc.tile_pool(name="vext", bufs=1) as vext_pool,
        ):
            masks_sb = consts.tile([128, 4, 512], BF16)
            nc.sync.dma_start(masks_sb[:], masks_d[:])
            ident_sb = consts.tile([128, 128], BF16)
            nc.sync.dma_start(ident_sb[:], ident_d[:])
            ones_sb = consts.tile([128, 64], F32R)
            nc.sync.dma_start(ones_sb[:], ones_d[:])

            # persistent SBUF tensors
            qk_sb = [qk_pool.tile([128, N], F32R, tag=f"qk{i}", name=f"qk{i}") for i in range(8)]
            v_ext = [vext_pool.tile([128, NT * 65], BF16, tag=f"ve{h}", name=f"ve{h}") for h in range(G)]
            # ---------------- Stage A: qkv projections (transposed) -------------
            with (
                tc.tile_pool(name="xT", bufs=1) as xT_pool,
                tc.tile_pool(name="wA", bufs=3) as w_pool,
                tc.tile_pool(name="vT", bufs=1) as vT_pool,
            ):
                xs = []
                for cc in range(CK):
                    t = xT_pool.tile([128, N], F32R, tag=f"x{cc}", name=f"x{cc}")
                    nc.sync.dma_start(t[:], xT_d[128 * cc:128 * (cc + 1), :])
                    xs.append(t)
                vT_sb = [vT_pool.tile([128, N], BF16, tag=f"v{i}", name=f"v{i}") for i in range(4)]

                with tc.tile_pool(name="psA", bufs=2, space="PSUM") as psA:
                  for ot in range(12):
                    wt = w_pool.tile([128, CK, 128], F32R)
                    for cc in range(CK):
                        nc.sync.dma_start(
                            wt[:, cc, :],
                            wqkvT_d[128 * cc:128 * (cc + 1), 128 * ot:128 * (ot + 1)],
                        )
                    ps = psA.tile([128, N], F32)
                    for cc in range(CK):
                        for ns in range(NS):
                            nc.tensor.matmul(
                                ps[:, 512 * ns:512 * (ns + 1)],
                                wt[:, cc, :],
                                xs[cc][:, 512 * ns:512 * (ns + 1)],
                                start=(cc == 0),
                                stop=(cc == CK - 1),
                            )
                    if ot < 8:
                        nc.scalar.copy(qk_sb[ot][:], ps[:])
                    else:
                        nc.vector.tensor_copy(vT_sb[ot - 8][:], ps[:])

                # ------------ Stage B: v_ext = transpose(v) + ones column -------
                with tc.tile_pool(name="psB", bufs=4, space="PSUM") as psB:
                    for h in range(G):
                        nc.vector.memset(v_ext[h][:], 1.0)
                    for vp in range(4):
                        for nt in range(NT):
                            tp = psB.tile([128, 128], BF16)
                            nc.tensor.transpose(
                                tp[:], vT_sb[vp][:, 128 * nt:128 * (nt + 1)], ident_sb[:]
                            )
                            ve3a = v_ext[2 * vp].rearrange("p (n e) -> p n e", e=65)
                            ve3b = v_ext[2 * vp + 1].rearrange("p (n e) -> p n e", e=65)
                            nc.vector.tensor_copy(ve3a[:, nt, 0:64], tp[:, 0:64])
                            nc.vector.tensor_copy(ve3b[:, nt, 0:64], tp[:, 64:128])

            # ---------------- Stage C: attention per head pair ------------------
            with tc.tile_pool(name="aoT", bufs=1) as aoT_pool:
              attn_outT = [aoT_pool.tile([128, N], F32R, tag=f"ao{p}", name=f"ao{p}") for p in range(4)]
              with (
                tc.tile_pool(name="pt", bufs=10) as pt_pool,
                tc.tile_pool(name="rr", bufs=3) as r_pool,
                tc.tile_pool(name="bcs", bufs=3) as bcs_pool,
                tc.tile_pool(name="tmpB", bufs=3) as tmpB_pool,
                tc.tile_pool(name="psS", bufs=2, space="PSUM") as psS,
                tc.tile_pool(name="psO", bufs=2, space="PSUM") as psO,
                tc.tile_pool(name="psBC", bufs=2, space="PSUM") as psBC,
              ):
                for p in range(4):
                    qT, kT = qk_sb[p], qk_sb[4 + p]
                    for s in range(NS):
                        nk = 4 * (s + 1)
                        oTs = []
                        for h in (0, 1):
                            hh = slice(64 * h, 64 * (h + 1))
                            hg = 2 * p + h
                            pts = []
                            for kg in range(nk // 2):
                                S2 = psS.tile([128, 1024], F32)
                                for j in (0, 1):
                                    k = 2 * kg + j
                                    nc.tensor.matmul(
                                        S2[:, 512 * j:512 * (j + 1)],
                                        kT[hh, 128 * k:128 * (k + 1)],
                                        qT[hh, 512 * s:512 * (s + 1)],
                                    )
                                P2 = pt_pool.tile([128, 1024], BF16, tag="pt")
                                nc.scalar.activation(P2[:], S2[:], EXP, scale=float(D) ** -0.5)
                                for j in (0, 1):
                                    k = 2 * kg + j
                                    if k >= 4 * s:
                                        ridx = k - 4 * s
                                        nc.vector.tensor_mul(
                                            P2[:, 512 * j:512 * (j + 1)],
                                            P2[:, 512 * j:512 * (j + 1)],
                                            masks_sb[:, ridx, :],
                                        )
                                pts.append(P2)
                            oT = psO.tile([65, 512], F32, tag="oT")
                            ve3 = v_ext[hg].rearrange("p (n e) -> p n e", e=65)
                            for kg in range(nk // 2):
                                for j in (0, 1):
                                    k = 2 * kg + j
                                    nc.tensor.matmul(
                                        oT[:],
                                        ve3[:, k, :],
                                        pts[kg][:, 512 * j:512 * (j + 1)],
                                        start=(k == 0),
                                        stop=(k == nk - 1),
                                    )
                            oTs.append(oT)
                        # normalize + assemble pair tile
                        for h in (0, 1):
                            oT = oTs[h]
                            Rf = r_pool.tile([128, 512], F32, tag="rf")
                            nc.vector.reciprocal(Rf[64:65, :], oT[64:65, :])
                            Rr = r_pool.tile([128, 512], F32R, tag="rr")
                            nc.vector.tensor_copy(Rr[64:65, :], Rf[64:65, :])
                            bc = psBC.tile([64, 512], F32, tag="bc")
                            nc.tensor.matmul(bc[:], ones_sb[64:65, :], Rr[64:65, :])
                            bcs = bcs_pool.tile([64, 512], F32, tag="bcs")
                            nc.vector.tensor_copy(bcs[:], bc[:])
                            if h == 0:
                                nc.vector.tensor_mul(
                                    attn_outT[p][0:64, 512 * s:512 * (s + 1)],
                                    oT[0:64, :],
                                    bcs[:],
                                )
                            else:
                                tmpB = tmpB_pool.tile([64, 512], F32R, tag="tb")
                                nc.vector.tensor_mul(tmpB[:], oT[0:64, :], bcs[:])
                                nc.sync.dma_start(
                                    attn_outT[p][64:128, 512 * s:512 * (s + 1)], tmpB[:]
                                )

              # ---------------- Stage D: output projection (partial) --------------
              with (
                tc.tile_pool(name="pj", bufs=1) as pj_pool,
                tc.tile_pool(name="ob", bufs=3) as ob_pool,
                tc.tile_pool(name="psD", bufs=3, space="PSUM") as psD,
            ):
                pj_sb = [pj_pool.tile([128, C], F32R, tag=f"pj{i}", name=f"pj{i}") for i in range(4)]
                for ac in range(4):
                    nc.sync.dma_start(pj_sb[ac][:], projT_d[128 * ac:128 * (ac + 1), :])
                for nt in range(NT):
                    pp = psD.tile([128, C], F32)
                    for ac in range(4):
                        for oc in (0, 1):
                            nc.tensor.matmul(
                                pp[:, 512 * oc:512 * (oc + 1)],
                                attn_outT[ac][:, 128 * nt:128 * (nt + 1)],
                                pj_sb[ac][:, 512 * oc:512 * (oc + 1)],
                                start=(ac == 0),
                                stop=(ac == 3),
                            )
                    ob = ob_pool.tile([128, C], F32)
                    nc.scalar.copy(ob[:], pp[:])
                    nc.sync.dma_start(out_d[128 * nt:128 * (nt + 1), :], ob[:])

    nc.compile()
    return nc


def _masks_np():
    import ml_dtypes

    m = np.zeros((128, 4, 512), dtype=np.float32)
    i = np.arange(128)[:, None]
    j = np.arange(512)[None, :]
    for ridx in range(4):
        m[:, ridx, :] = (j >= i + 128 * ridx).astype(np.float32)
    return m.astype(ml_dtypes.bfloat16)


def make_in_maps(x, qkv_w, proj_w):
    import ml_dtypes

    x = np.asarray(x, dtype=np.float32)
    qkv_w = np.asarray(qkv_w, dtype=np.float32)
    proj_w = np.asarray(proj_w, dtype=np.float32)
    masks = _masks_np()
    ident = np.eye(128, dtype=np.float32).astype(ml_dtypes.bfloat16)
    ones = np.ones((128, 64), dtype=np.float32)
    in_maps = []
    for c in range(8):
        b, g = c // 2, c % 2
        sl = slice(g * GC, (g + 1) * GC)
        wq, wk, wv = qkv_w[0:C][sl], qkv_w[C:2 * C][sl], qkv_w[2 * C:3 * C][sl]
        in_maps.append(
            {
                "xT": np.ascontiguousarray(x[b].T),
                "wqkvT": np.ascontiguousarray(np.concatenate([wq, wk, wv], 0).T),
                "projT": np.ascontiguousarray(proj_w[:, sl].T),
                "masks": masks,
                "ident": ident,
                "ones": ones,
            }
        )
    return in_maps


def kernel(x, qkv_w, proj_w, proj_b):
    proj_b = np.asarray(proj_b, dtype=np.float32)

    if "nc" not in _cache:
        _cache["nc"] = _build_nc()
    nc = _cache["nc"]

    in_maps = make_in_maps(x, qkv_w, proj_w)
    res = run_bass_kernel_spmd(nc, in_maps, core_ids=list(range(8)))
    out = np.stack(
        [res.results[2 * b]["out"] + res.results[2 * b + 1]["out"] for b in range(B)], 0
    )
    return (out + proj_b[None, None, :]).astype(np.float32)


# revision 8
# speedup vs baseline: 1.3386x; 1.3386x over previous
"""Causal attention layer (B=4, N=2048, C=1024, H=16, D=64) on 8 TRN2 NeuronCores.

Sharding: core c -> (batch b = c//2, head-group g = c%2 of 8 heads).
Per core, for its (b, g):
  qkvT  = wqkvT_g.T-contract(x_b)      q,k transposed [o,n]; v transposed then
                                       PE-transposed to [kn, 1|v] (ones col first)
  S_T   = kT.T @ qT                    pre-transposed scores [kn, qn], exp -> P_T bf16
  oT    = [1|v].T @ P_T                row 0 = softmax denominators, rows 1:65 = out.T
  attn_outT = oT[1:65] * bcast(1/oT[0])  (recip_approx_fast + gpsimd partition_broadcast)
  out_part  = attn_outT.T-contract(projT_g)
Host sums the two head-group partials per batch and adds proj_b.

Emission interleaves next-pair qkv matmuls and proj matmuls into the attention
stream as fillers so the PE stays dense (HAM-warm) while ScalarE runs exp.
"""
import sys

sys.path.insert(0, "/opt/trn_rl_repo")

import numpy as np

import concourse.bass as bass  # noqa: F401
import concourse.tile as tile
from concourse import bacc, mybir
from concourse.bass_utils import run_bass_kernel_spmd

F32 = mybir.dt.float32
F32R = mybir.dt.float32r
BF16 = mybir.dt.bfloat16
EXP = mybir.ActivationFunctionType.Exp

B, N, C, H, D = 4, 2048, 1024, 16, 64
G = 8            # heads per core
GC = G * D       # 512 channels per core
NT = N // 128    # 16 row tiles
NS = N // 512    # 4 row supers
CK = C // 128    # 8 contraction chunks

_cache = {}


def _build_nc():
    from contextlib import ExitStack

    nc = bacc.Bacc("TRN2", target_bir_lowering=False, debug=False)

    xT_d = nc.dram_tensor("xT", [C, N], F32R, kind="ExternalInput")
    wqkvT_d = nc.dram_tensor("wqkvT", [C, 3 * GC], F32R, kind="ExternalInput")
    projT_d = nc.dram_tensor("projT", [GC, C], F32R, kind="ExternalInput")
    tril_d = nc.dram_tensor("tril", [128, 128], BF16, kind="ExternalInput")
    ident_d = nc.dram_tensor("ident", [128, 128], BF16, kind="ExternalInput")
    out_d = nc.dram_tensor("out", [N, C], F32, kind="ExternalOutput")

    with tile.TileContext(nc) as tc:
        with ExitStack() as ctx:
            consts = ctx.enter_context(tc.tile_pool(name="consts", bufs=1))
            qk_pool = ctx.enter_context(tc.tile_pool(name="qk", bufs=4))
            vext_pool = ctx.enter_context(tc.tile_pool(name="vext", bufs=1))
            w_pool = ctx.enter_context(tc.tile_pool(name="wA", bufs=2))
            xT_pool = ctx.enter_context(tc.tile_pool(name="xT", bufs=1))
            rf_pool = ctx.enter_context(tc.tile_pool(name="rf", bufs=2))
            bcs_pool = ctx.enter_context(tc.tile_pool(name="bcs", bufs=2))
            tmp_pool = ctx.enter_context(tc.tile_pool(name="tmp", bufs=2))
            ob_pool = ctx.enter_context(tc.tile_pool(name="ob", bufs=2))
            pj_pool = ctx.enter_context(tc.tile_pool(name="pj", bufs=1))
            psA = ctx.enter_context(tc.tile_pool(name="psA", bufs=2, space="PSUM"))

            tril_sb = consts.tile([128, 128], BF16)
            nc.sync.dma_start(tril_sb[:], tril_d[:])
            ident_sb = consts.tile([128, 128], BF16)
            nc.sync.dma_start(ident_sb[:], ident_d[:])

            v_ext = [vext_pool.tile([128, NT * 65], BF16, tag=f"ve{h}", name=f"ve{h}")
                     for h in range(G)]
            ve3 = [v.rearrange("p (n e) -> p n e", e=65) for v in v_ext]

            xs = []
            for cc in range(CK):
                t = xT_pool.tile([128, N], F32R, tag=f"x{cc}", name=f"x{cc}")
                nc.sync.dma_start(t[:], xT_d[128 * cc:128 * (cc + 1), :])
                xs.append(t)

            pj_sb = [pj_pool.tile([128, C], F32R, tag=f"pj{i}", name=f"pj{i}")
                     for i in range(4)]
            for ac in range(4):
                nc.sync.dma_start(pj_sb[ac][:], projT_d[128 * ac:128 * (ac + 1), :])

            def load_w(ot):
                wt = w_pool.tile([128, CK, 128], F32R, tag="wt", name=f"wt{ot}")
                for cc in range(CK):
                    nc.sync.dma_start(
                        wt[:, cc, :],
                        wqkvT_d[128 * cc:128 * (cc + 1), 128 * ot:128 * (ot + 1)],
                    )
                return wt

            def qkv_quarter(wt, sup):
                psq = psA.tile([128, 512], F32, tag="qa", name="psq")
                for cc in range(CK):
                    nc.tensor.matmul(
                        psq[:],
                        wt[:, cc, :],
                        xs[cc][:, 512 * sup:512 * (sup + 1)],
                        start=(cc == 0),
                        stop=(cc == CK - 1),
                    )
                return psq

            # ---------------- Stage V: v projections + v_ext --------------------
            vT_ctx = ExitStack()
            vT_pool = vT_ctx.enter_context(tc.tile_pool(name="vT", bufs=1))
            vT_sb = [vT_pool.tile([128, N], BF16, tag=f"v{i}", name=f"v{i}")
                     for i in range(4)]
            for vp in range(4):
                wt = load_w(8 + vp)
                for sup in range(NS):
                    psq = qkv_quarter(wt, sup)
                    nc.vector.tensor_copy(vT_sb[vp][:, 512 * sup:512 * (sup + 1)], psq[:])
            for h in range(G):
                nc.vector.memset(v_ext[h][:], 1.0)
            with tc.tile_pool(name="psB", bufs=3, space="PSUM") as psB:
                for vp in range(4):
                    for nt in range(NT):
                        tp = psB.tile([128, 128], BF16, tag="tp", name="tp")
                        nc.tensor.transpose(
                            tp[:], vT_sb[vp][:, 128 * nt:128 * (nt + 1)], ident_sb[:]
                        )
                        nc.vector.tensor_copy(ve3[2 * vp][:, nt, 1:65], tp[:, 0:64])
                        nc.vector.tensor_copy(ve3[2 * vp + 1][:, nt, 1:65], tp[:, 64:128])
            vT_ctx.close()

            # ---------------- filler machinery ----------------------------------
            pending = []
            state = {"wt": None}

            def fill(n):
                for _ in range(n):
                    if not pending:
                        return
                    kind, fn = pending.pop(0)
                    if kind == "w":
                        state["wt"] = fn()
                    elif kind == "q":
                        fn(state["wt"])
                    else:
                        fn()

            def fill_all():
                while pending:
                    fill(1)

            def make_qk(p):
                qT = qk_pool.tile([128, N], F32R, tag="qk", name=f"q{p}")
                kT = qk_pool.tile([128, N], F32R, tag="qk", name=f"k{p}")
                steps = []
                for dst, ot in ((qT, p), (kT, 4 + p)):
                    def _load(ot=ot):
                        return load_w(ot)
                    steps.append(("w", _load))
                    for sup in range(NS):
                        def _mms(wt, dst=dst, sup=sup):
                            psq = qkv_quarter(wt, sup)
                            nc.vector.tensor_copy(
                                dst[:, 512 * sup:512 * (sup + 1)], psq[:]
                            )
                        steps.append(("q", _mms))
                return qT, kT, steps

            attn_outT = None  # set below; captured by proj closures

            def proj_steps(s):
                steps = []
                for nt in range(4 * s, 4 * s + 4):
                    for oc in (0, 1):
                        def _pj(nt=nt, oc=oc):
                            pp = psA.tile([128, 512], F32, tag="qa", name="pp")
                            for ac in range(4):
                                nc.tensor.matmul(
                                    pp[:],
                                    attn_outT[ac][:, 128 * nt:128 * (nt + 1)],
                                    pj_sb[ac][:, 512 * oc:512 * (oc + 1)],
                                    start=(ac == 0),
                                    stop=(ac == 3),
                                )
                            ob = ob_pool.tile([128, 512], F32, tag="ob", name="ob")
                            nc.vector.tensor_copy(ob[:], pp[:])
                            nc.sync.dma_start(
                                out_d[128 * nt:128 * (nt + 1),
                                      512 * oc:512 * (oc + 1)],
                                ob[:],
                            )
                        steps.append(("p", _pj))
                return steps

            # pair 0 q/k up front (nothing to hide it under)
            qT, kT, steps0 = make_qk(0)
            pending.extend(steps0)
            fill_all()

            # ---------------- attention pair loop (with fillers) ----------------
            with (
                tc.tile_pool(name="aoT", bufs=1) as aoT_pool,
                tc.tile_pool(name="pt", bufs=8) as pt_pool,
                tc.tile_pool(name="psS", bufs=2, space="PSUM") as psS,
                tc.tile_pool(name="psO", bufs=2, space="PSUM") as psO,
            ):
                attn_outT = [aoT_pool.tile([128, N], F32R, tag=f"ao{p}", name=f"ao{p}")
                             for p in range(4)]
                for p in range(4):
                    if p < 3:
                        nq, nk_, nsteps = make_qk(p + 1)
                        pending.extend(nsteps)
                    for s in range(NS):
                        nkb = 4 * (s + 1)
                        pts = {0: [], 1: []}
                        for kg in range(nkb // 2):
                            for h in (0, 1):
                                fill(2)
                                hh = slice(64 * h, 64 * (h + 1))
                                S2 = psS.tile([128, 1024], F32, tag="s2", name="S2")
                                for j in (0, 1):
                                    k = 2 * kg + j
                                    nc.tensor.matmul(
                                        S2[:, 512 * j:512 * (j + 1)],
                                        kT[hh, 128 * k:128 * (k + 1)],
                                        qT[hh, 512 * s:512 * (s + 1)],
                                    )
                                P2 = pt_pool.tile([128, 1024], BF16, tag="pt", name="P2")
                                nc.scalar.activation(
                                    P2[:], S2[:], EXP, scale=float(D) ** -0.5
                                )
                                for j in (0, 1):
                                    k = 2 * kg + j
                                    if k >= 4 * s:
                                        ridx = k - 4 * s
                                        c0 = 512 * j
                                        if ridx > 0:
                                            nc.vector.memset(
                                                P2[:, c0:c0 + 128 * ridx], 0.0
                                            )
                                        nc.vector.tensor_mul(
                                            P2[:, c0 + 128 * ridx:c0 + 128 * (ridx + 1)],
                                            P2[:, c0 + 128 * ridx:c0 + 128 * (ridx + 1)],
                                            tril_sb[:],
                                        )
                                pts[h].append(P2)
                        for h in (0, 1):
                            hg = 2 * p + h
                            oT = psO.tile([65, 512], F32, tag="oT", name="oT")
                            for kg in range(nkb // 2):
                                fill(1)
                                for j in (0, 1):
                                    k = 2 * kg + j
                                    nc.tensor.matmul(
                                        oT[:],
                                        ve3[hg][:, k, :],
                                        pts[h][kg][:, 512 * j:512 * (j + 1)],
                                        start=(k == 0),
                                        stop=(k == nkb - 1),
                                    )
                            Rf = rf_pool.tile([1, 512], F32, tag="rf", name="Rf")
                            nc.vector.reciprocal_approx_fast(Rf[:], oT[0:1, :])
                            bcs = bcs_pool.tile([65, 512], F32, tag="bcs", name="bcs")
                            nc.gpsimd.partition_broadcast(bcs[:], Rf[:])
                            tmp = tmp_pool.tile([65, 512], F32R, tag="tmp", name="tmp")
                            nc.vector.tensor_mul(tmp[:], oT[:], bcs[:])
                            nc.sync.dma_start(
                                attn_outT[p][64 * h:64 * (h + 1),
                                             512 * s:512 * (s + 1)],
                                tmp[1:65, :],
                            )
                        if p == 3:
                            pending.extend(proj_steps(s))
                            fill(2)
                    if p < 3:
                        qT, kT = nq, nk_
                fill_all()

    nc.compile()
    return nc


def _tril_np():
    import ml_dtypes

    i = np.arange(128)[:, None]
    j = np.arange(128)[None, :]
    return (j >= i).astype(np.float32).astype(ml_dtypes.bfloat16)


def make_in_maps(x, qkv_w, proj_w):
    import ml_dtypes

    x = np.asarray(x, dtype=np.float32)
    qkv_w = np.asarray(qkv_w, dtype=np.float32)
    proj_w = np.asarray(proj_w, dtype=np.float32)
    tril = _tril_np()
    ident = np.eye(128, dtype=np.float32).astype(ml_dtypes.bfloat16)
    in_maps = []
    for c in range(8):
        b, g = c // 2, c % 2
        sl = slice(g * GC, (g + 1) * GC)
        wq, wk, wv = qkv_w[0:C][sl], qkv_w[C:2 * C][sl], qkv_w[2 * C:3 * C][sl]
        in_maps.append(
            {
                "xT": np.ascontiguousarray(x[b].T),
                "wqkvT": np.ascontiguousarray(np.concatenate([wq, wk, wv], 0).T),
                "projT": np.ascontiguousarray(proj_w[:, sl].T),
                "tril": tril,
                "ident": ident,
            }
        )
    return in_maps


def kernel(x, qkv_w, proj_w, proj_b):
    proj_b = np.asarray(proj_b, dtype=np.float32)

    if "nc" not in _cache:
        _cache["nc"] = _build_nc()
    nc = _cache["nc"]

    in_maps = make_in_maps(x, qkv_w, proj_w)
    res = run_bass_kernel_spmd(nc, in_maps, core_ids=list(range(8)))
    out = np.stack(
        [res.results[2 * b]["out"] + res.results[2 * b + 1]["out"] for b in range(B)], 0
    )
    return (out + proj_b[None, None, :]).astype(np.float32)


# revision 16
# speedup vs baseline: 1.4304x; 1.0686x over previous
"""Causal attention layer (B=4, N=2048, C=1024, H=16, D=64) on 8 TRN2 NeuronCores.

Sharding: core c -> (batch b = c//2, head-group g = c%2 of 8 heads).
Per core, for its (b, g):
  qkvT  = wqkvT_g.T-contract(x_b)      q,k transposed [o,n]; v transposed then
                                       DMA-transposed to [kn, 1|v] (ones col first)
  S_T   = kT.T @ qT                    pre-transposed scores [kn, qn], exp -> P_T bf16
  oT    = [1|v].T @ P_T                row 0 = softmax denominators, rows 1:65 = out.T
  attn_outT = oT[1:65] * bcast(1/oT[0])  (recip_approx_fast + gpsimd partition_broadcast)
  out_part  = attn_outT.T-contract(projT_g)
Host sums the two head-group partials per batch and adds proj_b.

The emission stream is software-pipelined: only the first v o-tile and pair-0
q/k run up front; all later v o-tiles, v_ext transposes, q/k projections, and
the output projection are queued as fillers and interleaved into the attention
stream so the PE stays dense (HAM-warm) while ScalarE runs exp.
"""
import sys

sys.path.insert(0, "/opt/trn_rl_repo")

import numpy as np

import concourse.bass as bass  # noqa: F401
import concourse.tile as tile
from concourse import bacc, mybir
from concourse.bass_utils import run_bass_kernel_spmd

F32 = mybir.dt.float32
F32R = mybir.dt.float32r
BF16 = mybir.dt.bfloat16
EXP = mybir.ActivationFunctionType.Exp

B, N, C, H, D = 4, 2048, 1024, 16, 64
G = 8            # heads per core
GC = G * D       # 512 channels per core
NT = N // 128    # 16 row tiles
NS = N // 512    # 4 row supers
CK = C // 128    # 8 contraction chunks

_cache = {}


def _build_nc():
    from contextlib import ExitStack

    nc = bacc.Bacc("TRN2", target_bir_lowering=False, debug=False)

    xT_d = nc.dram_tensor("xT", [C, N], F32R, kind="ExternalInput")
    wqkvT_d = nc.dram_tensor("wqkvT", [C, 3 * GC], F32R, kind="ExternalInput")
    projT_d = nc.dram_tensor("projT", [GC, C], BF16, kind="ExternalInput")
    tril_d = nc.dram_tensor("tril", [128, 128], BF16, kind="ExternalInput")
    ident_d = nc.dram_tensor("ident", [128, 128], BF16, kind="ExternalInput")
    onesb_d = nc.dram_tensor("onesb", [128, NT], BF16, kind="ExternalInput")
    out_d = nc.dram_tensor("out", [N, C], F32, kind="ExternalOutput")

    with tile.TileContext(nc) as tc:
        with ExitStack() as ctx:
            consts = ctx.enter_context(tc.tile_pool(name="consts", bufs=1))
            qk_pool = ctx.enter_context(tc.tile_pool(name="qk", bufs=4))
            vext_pool = ctx.enter_context(tc.tile_pool(name="vext", bufs=1))
            vT_pool = ctx.enter_context(tc.tile_pool(name="vT", bufs=2))
            w_pool = ctx.enter_context(tc.tile_pool(name="wA", bufs=2))
            xT_pool = ctx.enter_context(tc.tile_pool(name="xT", bufs=1))
            rf_pool = ctx.enter_context(tc.tile_pool(name="rf", bufs=2))
            bcs_pool = ctx.enter_context(tc.tile_pool(name="bcs", bufs=2))
            tmp_pool = ctx.enter_context(tc.tile_pool(name="tmp", bufs=2))
            ob_pool = ctx.enter_context(tc.tile_pool(name="ob", bufs=2))
            pj_pool = ctx.enter_context(tc.tile_pool(name="pj", bufs=1))
            psA = ctx.enter_context(tc.tile_pool(name="psA", bufs=2, space="PSUM"))

            tril_sb = consts.tile([128, 128], BF16)
            nc.sync.dma_start(tril_sb[:], tril_d[:])
            ident_sb = consts.tile([128, 128], BF16)
            nc.sync.dma_start(ident_sb[:], ident_d[:])

            v_ext = [vext_pool.tile([128, NT * 128], BF16, tag=f"ve{h}", name=f"ve{h}")
                     for h in range(G)]
            ve3 = [v.rearrange("p (n e) -> p n e", e=128) for v in v_ext]
            for h in range(G):
                nc.sync.dma_start(ve3[h][:, :, 0:1], onesb_d[:, :, None])

            xs = []
            for cc in range(CK):
                t = xT_pool.tile([128, N], F32R, tag=f"x{cc}", name=f"x{cc}")
                nc.sync.dma_start(t[:], xT_d[128 * cc:128 * (cc + 1), :])
                xs.append(t)

            pj_sb = [pj_pool.tile([128, C], BF16, tag=f"pj{i}", name=f"pj{i}")
                     for i in range(4)]
            for ac in range(4):
                nc.sync.dma_start(pj_sb[ac][:], projT_d[128 * ac:128 * (ac + 1), :])

            def load_w(ot):
                wt = w_pool.tile([128, CK, 128], F32R, tag="wt", name=f"wt{ot}")
                src = wqkvT_d[:, 128 * ot:128 * (ot + 1)].rearrange(
                    "(cc p) o -> p cc o", p=128
                )
                nc.sync.dma_start(wt[:], src)
                return wt

            def qkv_quarter(wt, sup):
                psq = psA.tile([128, 512], F32, tag="qa", name="psq")
                for cc in range(CK):
                    nc.tensor.matmul(
                        psq[:],
                        wt[:, cc, :],
                        xs[cc][:, 512 * sup:512 * (sup + 1)],
                        start=(cc == 0),
                        stop=(cc == CK - 1),
                    )
                return psq

            tr_insts = [[None] * NT for _ in range(G)]

            # ------- step builders (emitted inline or queued as fillers) --------
            def v_steps(vp):
                """v o-tile vp -> vT (bf16) -> DMA-transpose into v_ext[2vp,2vp+1]."""
                vt = vT_pool.tile([128, N], BF16, tag="vt", name=f"vt{vp}")
                cps = [None] * NS
                steps = [("w", lambda vp=vp: load_w(8 + vp))]
                for sup in range(NS):
                    def _mms(wt, vt=vt, sup=sup):
                        psq = qkv_quarter(wt, sup)
                        cps[sup] = nc.vector.tensor_copy(
                            vt[:, 512 * sup:512 * (sup + 1)], psq[:]
                        )
                    steps.append(("q", _mms))
                for nt in range(NT):
                    def _tr(vt=vt, vp=vp, nt=nt):
                        tp = psA.tile([128, 128], BF16, tag="qa", name="tp")
                        nc.tensor.transpose(
                            tp[:], vt[:, 128 * nt:128 * (nt + 1)], ident_sb[:]
                        )
                        nc.vector.tensor_copy(
                            ve3[2 * vp][:, nt, 64:128], tp[:, 0:64]
                        )
                        nc.vector.tensor_copy(
                            ve3[2 * vp + 1][:, nt, 64:128], tp[:, 64:128]
                        )
                    steps.append(("p", _tr))
                return steps

            def qk_steps(p):
                qT = qk_pool.tile([128, N], F32R, tag="qk", name=f"q{p}")
                kT = qk_pool.tile([128, N], F32R, tag="qk", name=f"k{p}")
                steps = []
                for dst, ot in ((qT, p), (kT, 4 + p)):
                    steps.append(("w", lambda ot=ot: load_w(ot)))
                    for sup in range(NS):
                        def _mms(wt, dst=dst, sup=sup):
                            psq = qkv_quarter(wt, sup)
                            nc.vector.tensor_copy(
                                dst[:, 512 * sup:512 * (sup + 1)], psq[:]
                            )
                        steps.append(("q", _mms))
                return qT, kT, steps

            attn_outT = None

            def proj_steps(s):
                steps = []
                for nt in range(4 * s, 4 * s + 4):
                    for oc in (0, 1):
                        def _pj(nt=nt, oc=oc):
                            pp = psA.tile([128, 512], F32, tag="qa", name="pp")
                            for ac in range(4):
                                nc.tensor.matmul(
                                    pp[:],
                                    attn_outT[ac][:, 128 * nt:128 * (nt + 1)],
                                    pj_sb[ac][:, 512 * oc:512 * (oc + 1)],
                                    start=(ac == 0),
                                    stop=(ac == 3),
                                )
                            ob = ob_pool.tile([128, 512], F32, tag="ob", name="ob")
                            nc.vector.tensor_copy(ob[:], pp[:])
                            nc.sync.dma_start(
                                out_d[128 * nt:128 * (nt + 1),
                                      512 * oc:512 * (oc + 1)],
                                ob[:],
                            )
                        steps.append(("p", _pj))
                return steps

            # filler machinery: fill(n) emits until n PE-carrying steps are out
            pending = []
            state = {"wt": None}

            def fill(n):
                done = 0
                while pending and done < n:
                    kind, fn = pending.pop(0)
                    if kind == "w":
                        state["wt"] = fn()
                    elif kind == "q":
                        fn(state["wt"])
                        done += 1
                    elif kind == "t":
                        fn()
                    else:
                        fn()
                        done += 1

            def fill_all():
                while pending:
                    fill(4)

            # ---------------- prologue: vp0 + pair-0 q/k ------------------------
            for kind, fn in v_steps(0):
                if kind == "w":
                    state["wt"] = fn()
                elif kind == "q":
                    fn(state["wt"])
                else:
                    fn()
            qT, kT, steps0 = qk_steps(0)
            pending.extend(steps0)
            fill_all()

            # ---------------- attention pair loop (with fillers) ----------------
            with (
                tc.tile_pool(name="aoT", bufs=1) as aoT_pool,
                tc.tile_pool(name="pt", bufs=7) as pt_pool,
                tc.tile_pool(name="psS", bufs=2, space="PSUM") as psS,
                tc.tile_pool(name="psO", bufs=2, space="PSUM") as psO,
            ):
                attn_outT = [aoT_pool.tile([128, N], BF16, tag=f"ao{p}", name=f"ao{p}")
                             for p in range(4)]
                for p in range(4):
                    if p < 3:
                        pending.extend(v_steps(p + 1))
                        nq, nk_, nsteps = qk_steps(p + 1)
                        pending.extend(nsteps)

                    for s in range(NS):
                        nkb = 4 * (s + 1)
                        pts = {0: [], 1: []}
                        for kg in range(nkb // 2):
                            for h in (0, 1):
                                fill(2)
                                hh = slice(64 * h, 64 * (h + 1))
                                S2 = psS.tile([128, 1024], F32, tag="s2", name="S2")
                                for j in (0, 1):
                                    k = 2 * kg + j
                                    nc.tensor.matmul(
                                        S2[:, 512 * j:512 * (j + 1)],
                                        kT[hh, 128 * k:128 * (k + 1)],
                                        qT[hh, 512 * s:512 * (s + 1)],
                                    )
                                P2 = pt_pool.tile([128, 1024], BF16, tag="pt", name="P2")
                                nc.scalar.activation(
                                    P2[:], S2[:], EXP, scale=float(D) ** -0.5
                                )
                                for j in (0, 1):
                                    k = 2 * kg + j
                                    if k >= 4 * s:
                                        ridx = k - 4 * s
                                        c0 = 512 * j
                                        if ridx > 0:
                                            nc.vector.memset(
                                                P2[:, c0:c0 + 128 * ridx], 0.0
                                            )
                                        nc.vector.tensor_mul(
                                            P2[:, c0 + 128 * ridx:c0 + 128 * (ridx + 1)],
                                            P2[:, c0 + 128 * ridx:c0 + 128 * (ridx + 1)],
                                            tril_sb[:],
                                        )
                                pts[h].append(P2)
                        for h in (0, 1):
                            hg = 2 * p + h
                            oT = psO.tile([128, 512], F32, tag="oT", name="oT")
                            for kg in range(nkb // 2):
                                fill(1)
                                for j in (0, 1):
                                    k = 2 * kg + j
                                    nc.tensor.matmul(
                                        oT[:],
                                        ve3[hg][:, k, :],
                                        pts[h][kg][:, 512 * j:512 * (j + 1)],
                                        start=(k == 0),
                                        stop=(k == nkb - 1),
                                    )
                            Rf = rf_pool.tile([1, 512], F32, tag="rf", name="Rf")
                            nc.vector.reciprocal_approx_fast(Rf[:], oT[0:1, :])
                            bcs = bcs_pool.tile([128, 512], F32, tag="bcs", name="bcs")
                            nc.gpsimd.partition_broadcast(bcs[:], Rf[:])
                            tmp = tmp_pool.tile([128, 512], BF16, tag="tmp", name="tmp")
                            nc.vector.tensor_mul(tmp[:], oT[:], bcs[:])
                            nc.sync.dma_start(
                                attn_outT[p][64 * h:64 * (h + 1),
                                             512 * s:512 * (s + 1)],
                                tmp[64:128, :],
                            )
                        if p == 3:
                            pending.extend(proj_steps(s))
                            fill(2)
                    if p < 3:
                        qT, kT = nq, nk_
                fill_all()

    nc.compile()
    return nc


def _tril_np():
    import ml_dtypes

    i = np.arange(128)[:, None]
    j = np.arange(128)[None, :]
    return (j >= i).astype(np.float32).astype(ml_dtypes.bfloat16)


def make_in_maps(x, qkv_w, proj_w):
    x = np.asarray(x, dtype=np.float32)
    qkv_w = np.asarray(qkv_w, dtype=np.float32)
    proj_w = np.asarray(proj_w, dtype=np.float32)
    tril = _tril_np()
    in_maps = []
    for c in range(8):
        b, g = c // 2, c % 2
        sl = slice(g * GC, (g + 1) * GC)
        wq, wk, wv = qkv_w[0:C][sl], qkv_w[C:2 * C][sl], qkv_w[2 * C:3 * C][sl]
        in_maps.append(
            {
                "xT": np.ascontiguousarray(x[b].T),
                "wqkvT": np.ascontiguousarray(np.concatenate([wq, wk, wv], 0).T),
                "projT": np.ascontiguousarray(proj_w[:, sl].T).astype(
                    __import__("ml_dtypes").bfloat16
                ),
                "tril": tril,
                "ident": np.eye(128, dtype=np.float32).astype(
                    __import__("ml_dtypes").bfloat16
                ),
                "onesb": np.ones((128, NT), dtype=np.float32).astype(
                    __import__("ml_dtypes").bfloat16
                ),
            }
        )
    return in_maps


def kernel(x, qkv_w, proj_w, proj_b):
    proj_b = np.asarray(proj_b, dtype=np.float32)

    if "nc" not in _cache:
        _cache["nc"] = _build_nc()
    nc = _cache["nc"]

    in_maps = make_in_maps(x, qkv_w, proj_w)
    res = run_bass_kernel_spmd(nc, in_maps, core_ids=list(range(8)))
    out = np.stack(
        [res.results[2 * b]["out"] + res.results[2 * b + 1]["out"] for b in range(B)], 0
    )
    return (out + proj_b[None, None, :]).astype(np.float32)
